# revision 12
# baseline (speedup 1.0000x reference)
"""Distributed Trainium2 kernel for MinkowskiEngine-style sparse transposed
conv + BatchNorm + ReLU (gather -> per-offset GEMM -> scatter-add -> BN -> ReLU).

Strategy (8 NeuronCores, SPMD):
  - Owner-partition the 200k output rows: core c owns rows [c*25000, (c+1)*25000).
    Every edge (k, m) is routed on the host to the core owning out_map[k, m].
  - Per core, the 25000 owned rows are split into 4 blocks of 6250. Edges are
    grouped by (block, x-chunk, k); x is staged as bf16 and gathered with the
    hardware transpose-gather, which directly yields the [C_in, m] stationary
    operand for the TensorEngine (no on-chip transposes anywhere).
  - Form-A GEMM: per 128-edge tile, lhsT = gathered-transposed x slice,
    rhs = W_k K-tile; PSUM accumulates the 2 K-tiles; messages land
    partition-major and are copied (f32->bf16) into a per-block SBUF message
    buffer laid out as gather tokens.
  - Scatter-add without races: the host sorts each block's rows by incoming
    message count (descending); round r adds the r-th message of every row
    that has one -- a contiguous prefix of the block's columns -- via an
    SBUF-source gather (channel-major) + one DVE add. Padding slots point at
    a zeroed token rank.
  - Block accumulators spill to DRAM; per-channel sum/sumsq reduce on the fly;
    one tiny [128,2] AllReduce gives global BatchNorm stats (sync-BN); the
    normalize+ReLU pass streams the raw output back through the ACT engine
    with per-partition scale/bias fused into a single activation op.
  - The host-side permutation (count-sorted rows, channel-major layout) is
    inverted for free during unsharding.
"""
import sys

sys.path.insert(0, "/opt/trn_rl_repo")

import numpy as np
import ml_dtypes

from concourse import bass, mybir, bacc
from concourse import tile
from concourse.bass_utils import run_bass_kernel_spmd

F32 = mybir.dt.float32
BF16 = mybir.dt.bfloat16
I16 = mybir.dt.int16

# problem constants (hardcoded per harness contract)
N_IN = 100000
N_OUT = 200000
C_IN = 256
C_OUT = 128
K = 8
NC = 8
RPC = N_OUT // NC          # rows per core = 25000
B = 4                      # blocks per core
RPB = RPC // B             # rows per block = 6250
XCH = 4                    # x chunks (int16 gather index limit)
XCHSZ = N_IN // XCH        # 25000
COLSTRIDE = 6272           # block column stride in acc (>= RPB, mult of 128)
NCOLS = B * COLSTRIDE      # 25088
BN_EPS = 1e-5
GCHUNK = 896               # max indices per transpose dma_gather call (HW
                           # descriptor-ring limit: >=1024 wedges the device)
NORMCH = 1568              # normalize/stat chunk columns
NQ = 1                     # SWDGE queues

LAST_EXEC_NS = [None]


def _ceil(a, b):
    return (a + b - 1) // b


def _wrap_idx(arr):
    """[n] -> [128, n/16] wrapped+replicated layout for dma_gather/scatter."""
    n = arr.shape[0]
    assert n % 16 == 0
    w = arr.reshape(n // 16, 16).T.astype(np.int16)
    return np.tile(w, (8, 1))


def _preprocess(in_map, out_map):
    """Route edges to owner cores and build all static structure + per-core
    index tensors."""
    im = in_map.reshape(-1).astype(np.int64)
    dm = out_map.reshape(-1).astype(np.int64)
    ne = im.shape[0]
    kk = np.arange(ne, dtype=np.int64) // N_IN

    core = dm // RPC
    loc = dm - core * RPC
    blk = loc // RPB
    row = loc - blk * RPB
    xch = im // XCHSZ
    xli = im - xch * XCHSZ

    # group id in (core, blk, xch, k) order: xch outer of k so one gather call
    # covers several k sub-segments from a single x chunk
    gid = ((core * B + blk) * XCH + xch) * K + kk
    order = np.argsort(gid, kind="stable")
    gcnt = np.bincount(gid, minlength=NC * B * XCH * K).reshape(NC, B, XCH, K)

    # static (cross-core max) group sizes, padded to 128
    S = (_ceil(np.maximum(gcnt.max(axis=0), 1), 128) * 128).astype(np.int64)  # [B,XCH,K]
    grp_off = np.zeros((B, XCH, K), np.int64)  # slot offset within block stream
    goff = np.zeros((B, XCH, K), np.int64)     # gather-idx offset within block
    for b in range(B):
        off = 0
        for c4 in range(XCH):
            for k in range(K):
                grp_off[b, c4, k] = off
                goff[b, c4, k] = off
                off += S[b, c4, k]
    GB = S.reshape(B, -1).sum(axis=1)          # padded gather slots per block
    assert int(GB.max()) + 128 <= 255 * 128, "msg token ids exceed int16 range"
    RANKS = int(GB.max()) // 128 + 1           # msg buffer ranks (rank 0 = zeros)

    # per-core, per-block edge data (sorted by gid, stable)
    im_s = xli[order]
    row_s = row[order]
    core_s = core[order]
    blk_s = blk[order]
    gid_s = gid[order]

    # position within group
    grp_start = np.zeros(NC * B * XCH * K + 1, np.int64)
    np.cumsum(np.bincount(gid_s, minlength=NC * B * XCH * K), out=grp_start[1:])
    pos_in_grp = np.arange(ne) - grp_start[gid_s]

    # message slot (token id) of each edge
    b_of = blk_s
    c4_of = (gid_s // K) % XCH
    k_of = gid_s % K
    slot = 128 + grp_off[b_of, c4_of, k_of] + pos_in_grp

    # ---- rounds (static sizes = cross-core max) -------------------------
    # per (core, block): counts per row, rows sorted by count desc
    cb = core_s * B + blk_s
    counts = np.zeros((NC * B, RPB), np.int64)
    np.add.at(counts, (cb, row_s), 1)
    maxcnt = int(counts.max())
    # n_r[cb, r] = #rows with count > r
    cnt_sorted = -np.sort(-counts, axis=1)
    n_r = np.zeros((NC * B, maxcnt), np.int64)
    for r in range(maxcnt):
        n_r[:, r] = (cnt_sorted > r).sum(axis=1)
    NR = n_r.reshape(NC, B, maxcnt).max(axis=0)  # [B, maxcnt] static round sizes
    ROUNDS = [[int(NR[b, r]) for r in range(maxcnt) if NR[b, r] > 0] for b in range(B)]
    RT = [(_ceil(max(sum(rs), 1), 128) * 128) for rs in ROUNDS]  # padded ridx len/block

    # row rank within block (count-sorted, stable by row id)
    rank_of_row = np.zeros((NC * B, RPB), np.int64)
    rows_sorted = np.argsort(-counts, axis=1, kind="stable")
    for i in range(NC * B):
        rank_of_row[i, rows_sorted[i]] = np.arange(RPB)

    # occurrence index of each edge within its row (in slot order = gid order)
    key = cb * RPB + row_s
    okey = np.argsort(key, kind="stable")
    kstart = np.zeros(NC * B * RPB + 1, np.int64)
    np.cumsum(np.bincount(key, minlength=NC * B * RPB), out=kstart[1:])
    occ = np.empty(ne, np.int64)
    occ[okey] = np.arange(ne) - kstart[key[okey]]

    # round src offsets within the block's ridx stream
    r_off = [np.concatenate([[0], np.cumsum(rs)]).astype(np.int64) for rs in ROUNDS]

    # ---- build per-core tensors ----------------------------------------
    gidx_cores, ridx_cores = [], []
    for c in range(NC):
        gidx = np.zeros((B, int(GB.max())), np.int64)
        for b in range(B):
            sel = (core_s == c) & (blk_s == b)
            gidx[b, goff[b, c4_of[sel], k_of[sel]] + pos_in_grp[sel]] = im_s[sel]
            # padding gathers row 0 of the chunk (garbage msgs, never read)
        ridx = np.zeros((B, max(RT)), np.int64)  # 0 = zero token
        sel = core_s == c
        bsel = blk_s[sel]
        qsel = rank_of_row[cb[sel], row_s[sel]]
        rsel = occ[sel]
        ssel = slot[sel]
        roffs = np.array([[r_off[b][r] if r < len(ROUNDS[b]) else 0
                           for r in range(maxcnt + 1)] for b in range(B)])
        ridx[bsel, roffs[bsel, rsel] + qsel] = ssel
        gidx_cores.append(np.concatenate([_wrap_idx(gidx[b]) for b in range(B)], axis=1))
        ridx_cores.append(np.concatenate([_wrap_idx(ridx[b]) for b in range(B)], axis=1))

    # per-block map: 128-slot tile index -> k (for weight selection)
    tile_k = []
    for b in range(B):
        tk = []
        for c4 in range(XCH):
            for k in range(K):
                tk += [k] * (int(S[b, c4, k]) // 128)
        tile_k.append(tk)

    meta = dict(S=S, goff=goff, GB=GB, RANKS=RANKS, ROUNDS=ROUNDS, RT=RT,
                rows_sorted=rows_sorted, maxcnt=maxcnt, tile_k=tile_k)
    return gidx_cores, ridx_cores, meta


def _build(meta, sim_mode=False):
    S, goff, GB, RANKS = meta["S"], meta["goff"], meta["GB"], meta["RANKS"]
    ROUNDS, RT = meta["ROUNDS"], meta["RT"]
    tile_k = meta["tile_k"]
    GBmax = int(GB.max())
    RTmax = max(RT)

    nc = bacc.Bacc(num_devices=NC, num_swdge_queues=NQ)
    x_d = nc.dram_tensor("x", [N_IN, C_IN], BF16, kind="ExternalInput")
    w_d = nc.dram_tensor("w", [128, 2 * K, C_OUT], BF16, kind="ExternalInput")
    gidx_d = nc.dram_tensor("gidx", [128, B * GBmax // 16], I16, kind="ExternalInput")
    ridx_d = nc.dram_tensor("ridx", [128, B * RTmax // 16], I16, kind="ExternalInput")
    gamma_d = nc.dram_tensor("gamma", [C_OUT, 1], F32, kind="ExternalInput")
    beta_d = nc.dram_tensor("beta", [C_OUT, 1], F32, kind="ExternalInput")
    out_d = nc.dram_tensor("out", [128, NCOLS], F32, kind="ExternalOutput")
    raw_d = nc.dram_tensor("raw", [128, NCOLS], F32)
    cc_in = nc.dram_tensor("cc_in", [128, 2], F32)
    cc_out = nc.dram_tensor("cc_out", [128, 2], F32)

    with tile.TileContext(nc) as tc:
        with (
            tc.tile_pool(name="const", bufs=1) as cpool,
            tc.tile_pool(name="accp", bufs=1) as apool,
            tc.tile_pool(name="msgp", bufs=1) as mpool,
            tc.tile_pool(name="xgp", bufs=6) as xpool,
            tc.tile_pool(name="idxp", bufs=2) as ipool,
            tc.tile_pool(name="rgp", bufs=4) as rpool,
            tc.tile_pool(name="tmpp", bufs=2) as tpool,
            tc.tile_pool(name="psum", bufs=4, space="PSUM") as pspool,
        ):
            w_sb = cpool.tile([128, 2 * K, C_OUT], BF16)
            nc.sync.dma_start(w_sb[:], w_d[:])
            gamma_sb = cpool.tile([C_OUT, 1], F32)
            nc.sync.dma_start(gamma_sb[:], gamma_d[:])
            beta_sb = cpool.tile([C_OUT, 1], F32)
            nc.sync.dma_start(beta_sb[:], beta_d[:])
            ssum = cpool.tile([128, 1], F32)
            nc.vector.memset(ssum[:], 0.0)
            ssq = cpool.tile([128, 1], F32)
            nc.vector.memset(ssq[:], 0.0)

            for b in range(B):
                gidx_sb = ipool.tile([128, GBmax // 16], I16, tag="gidx")
                nc.sync.dma_start(
                    gidx_sb[:], gidx_d[:, b * (GBmax // 16):(b + 1) * (GBmax // 16)]
                )
                ridx_sb = ipool.tile([128, RTmax // 16], I16, tag="ridx")
                nc.sync.dma_start(
                    ridx_sb[:], ridx_d[:, b * (RTmax // 16):(b + 1) * (RTmax // 16)]
                )

                msgbuf = mpool.tile([128, RANKS, C_OUT], BF16, tag="msg")
                nc.vector.memset(msgbuf[:, 0, :], 0.0)

                # ---- gather + GEMM (chunked calls, <=GCHUNK idxs) -----
                for c4 in range(XCH):
                    xsrc = x_d[c4 * XCHSZ:(c4 + 1) * XCHSZ, :]
                    seg_off = int(goff[b, c4, 0])
                    seg_len = int(S[b, c4, :].sum())
                    pos = 0
                    while pos < seg_len:
                        ln = min(GCHUNK, seg_len - pos)
                        off = seg_off + pos
                        xgT = xpool.tile([128, 2, ln], BF16, tag="xg")
                        nc.gpsimd.dma_gather(
                            out_ap=xgT[:],
                            in_ap=xsrc,
                            idxs_ap=gidx_sb[:, off // 16:(off + ln) // 16],
                            num_idxs=ln,
                            num_idxs_reg=ln,
                            elem_size=C_IN,
                            transpose=True,
                        )
                        t = 0
                        while t < ln // 128:
                            nt = min(4, ln // 128 - t)
                            ps = pspool.tile([128, 512], F32, tag="ps")
                            for j in range(nt):
                                gt = off // 128 + t + j  # global tile in block
                                k = tile_k[b][gt]
                                sl = slice((t + j) * 128, (t + j + 1) * 128)
                                nc.tensor.matmul(
                                    ps[:, j * 128:(j + 1) * 128],
                                    xgT[:, 0, sl], w_sb[:, 2 * k, :],
                                    start=True, stop=False,
                                )
                                nc.tensor.matmul(
                                    ps[:, j * 128:(j + 1) * 128],
                                    xgT[:, 1, sl], w_sb[:, 2 * k + 1, :],
                                    start=False, stop=True,
                                )
                            rank0 = 1 + off // 128 + t
                            nc.vector.tensor_copy(
                                msgbuf[:, rank0:rank0 + nt, :],
                                ps[:, :nt * 128],
                            )
                            t += nt
                        pos += ln

                # ---- rounds: gather msgs + prefix adds ----------------
                acc = apool.tile([128, COLSTRIDE], F32, tag="acc")
                nc.vector.memset(acc[:], 0.0)
                segs = []  # (src_off, dst_off, len) in round space
                src = 0
                for n in ROUNDS[b]:
                    segs.append((src, 0, n))
                    src += n
                total = RT[b]
                msgraw = msgbuf[:].bitcast(mybir.dt.uint8)
                gpos = 0
                while gpos < total:
                    gl = min(GCHUNK, total - gpos)
                    g = rpool.tile([128, 1, gl], BF16, tag="rg")
                    nc.gpsimd.dma_gather(
                        out_ap=g[:],
                        in_ap=msgraw,
                        idxs_ap=ridx_sb[:, gpos // 16:(gpos + gl) // 16],
                        num_idxs=gl,
                        num_idxs_reg=gl,
                        elem_size=C_OUT,
                        transpose=True,
                        sbuf_tokens_per_rank=128,
                        sbuf_free_dim_per_rank=C_OUT * 2,
                    )
                    for (soff, doff, slen) in segs:
                        lo = max(soff, gpos)
                        hi = min(soff + slen, gpos + gl)
                        if lo >= hi:
                            continue
                        d0 = doff + (lo - soff)
                        nc.vector.tensor_tensor(
                            acc[:, d0:d0 + hi - lo],
                            acc[:, d0:d0 + hi - lo],
                            g[:, 0, lo - gpos:hi - gpos],
                            op=mybir.AluOpType.add,
                        )
                    gpos += gl

                # ---- per-block stats + spill --------------------------
                r1 = tpool.tile([128, 1], F32, tag="r1")
                nc.vector.reduce_sum(r1[:], acc[:], mybir.AxisListType.X)
                nc.vector.tensor_tensor(ssum[:], ssum[:], r1[:], op=mybir.AluOpType.add)
                for i in range(0, COLSTRIDE, NORMCH):
                    sq = tpool.tile([128, NORMCH], F32, tag="sq")
                    nc.scalar.square(sq[:], acc[:, i:i + NORMCH])
                    r2 = tpool.tile([128, 1], F32, tag="r2")
                    nc.vector.reduce_sum(r2[:], sq[:], mybir.AxisListType.X)
                    nc.vector.tensor_tensor(ssq[:], ssq[:], r2[:], op=mybir.AluOpType.add)
                nc.sync.dma_start(raw_d[:, b * COLSTRIDE:(b + 1) * COLSTRIDE], acc[:])

            # ---- global BN stats (sync-BN AllReduce) ------------------
            st = cpool.tile([128, 2], F32)
            nc.vector.tensor_copy(st[:, 0:1], ssum[:])
            nc.vector.tensor_copy(st[:, 1:2], ssq[:])
            nc.sync.dma_start(cc_in[:], st[:])
            if sim_mode:
                nc.sync.dma_start(cc_out[:], cc_in[:])
            else:
                nc.gpsimd.collective_compute(
                    "AllReduce", mybir.AluOpType.add,
                    replica_groups=[list(range(NC))],
                    ins=[cc_in[:]], outs=[cc_out[:]],
                )
            st2 = cpool.tile([128, 2], F32)
            nc.sync.dma_start(st2[:], cc_out[:])

            mean = cpool.tile([128, 1], F32)
            nc.scalar.mul(mean[:], st2[:, 0:1], 1.0 / N_OUT)
            e2 = cpool.tile([128, 1], F32)
            nc.scalar.mul(e2[:], st2[:, 1:2], 1.0 / N_OUT)
            m2 = cpool.tile([128, 1], F32)
            nc.scalar.square(m2[:], mean[:])
            var = cpool.tile([128, 1], F32)
            nc.vector.tensor_sub(var[:], e2[:], m2[:])
            eps_sb = cpool.tile([128, 1], F32)
            nc.vector.memset(eps_sb[:], BN_EPS)
            std = cpool.tile([128, 1], F32)
            nc.scalar.activation(std[:], var[:], mybir.ActivationFunctionType.Sqrt,
                                 bias=eps_sb[:], scale=1.0)
            inv = cpool.tile([128, 1], F32)
            nc.vector.reciprocal(inv[:], std[:])
            scl = cpool.tile([128, 1], F32)
            nc.vector.tensor_mul(scl[:], inv[:], gamma_sb[:])
            ms = cpool.tile([128, 1], F32)
            nc.vector.tensor_mul(ms[:], mean[:], scl[:])
            bia = cpool.tile([128, 1], F32)
            nc.vector.tensor_sub(bia[:], beta_sb[:], ms[:])

            # ---- normalize + ReLU + store -----------------------------
            for i in range(0, NCOLS, NORMCH):
                raw = tpool.tile([128, NORMCH], F32, tag="nr")
                nc.sync.dma_start(raw[:], raw_d[:, i:i + NORMCH])
                y = tpool.tile([128, NORMCH], F32, tag="ny")
                nc.scalar.activation(y[:], raw[:],
                                     mybir.ActivationFunctionType.Relu,
                                     bias=bia[:], scale=scl[:])
                nc.sync.dma_start(out_d[:, i:i + NORMCH], y[:])

    # Route each SWDGE custom op to the queue matching its Tile-assigned
    # DMASW lane (sem lane i is claimed by queue i % NQ; a mismatched queue
    # incrementing the lane's sem is rejected by ucode/sim).
    from concourse.tile_sem_assignment import PROC_NAME_TO_IDX
    dmasw = {PROC_NAME_TO_IDX[f"DMASW{i}"]: i for i in range(8)}
    for ins in nc.inst_map.values():
        if isinstance(ins, mybir.InstDMAGatherAnt):
            proc = getattr(ins, "bass_scheduled_proc", None)
            if proc in dmasw:
                ins.queue_num = dmasw[proc] % NQ if NQ > 1 else 0

    nc.compile()
    return nc


def kernel(x_feats, weight, gamma, beta, in_map, out_map, n_out, _trace=False):
    assert int(n_out) == N_OUT
    gidx_cores, ridx_cores, meta = _preprocess(np.asarray(in_map), np.asarray(out_map))
    nc = _build(meta)

    xbf = np.asarray(x_feats, np.float32).astype(ml_dtypes.bfloat16)
    wbf = np.asarray(weight, np.float32).astype(ml_dtypes.bfloat16)
    # w[k, t*128+p, c] -> w_sb[p, 2k+t, c]
    wdev = np.ascontiguousarray(
        wbf.reshape(K, 2, 128, C_OUT).transpose(2, 0, 1, 3).reshape(128, 2 * K, C_OUT)
    )
    gdev = np.asarray(gamma, np.float32).reshape(C_OUT, 1)
    bdev = np.asarray(beta, np.float32).reshape(C_OUT, 1)

    in_maps = []
    for c in range(NC):
        in_maps.append({
            "x": xbf,
            "w": wdev,
            "gidx": np.ascontiguousarray(gidx_cores[c]),
            "ridx": np.ascontiguousarray(ridx_cores[c]),
            "gamma": gdev,
            "beta": bdev,
        })

    kw = {}
    if _trace:
        kw = dict(trace=True)
    res = run_bass_kernel_spmd(nc, in_maps, core_ids=list(range(NC)), **kw)
    LAST_EXEC_NS[0] = res.exec_time_ns

    # ---- unshard: invert channel-major layout + count-sorted row perm ----
    out = np.empty((N_OUT, C_OUT), np.float32)
    rows_sorted = meta["rows_sorted"]  # [NC*B, RPB]
    for c in range(NC):
        y = res.results[c]["out"]  # [128, NCOLS]
        for b in range(B):
            cols = y[:, b * COLSTRIDE: b * COLSTRIDE + RPB]  # [128, RPB]
            rows = c * RPC + b * RPB + rows_sorted[c * B + b]
            out[rows] = cols.T
    return out


# revision 16
# speedup vs baseline: 1.7088x; 1.7088x over previous
"""Distributed Trainium2 kernel for MinkowskiEngine-style sparse transposed
conv + BatchNorm + ReLU (gather -> per-offset GEMM -> scatter-add -> BN -> ReLU).

v2 strategy (8 NeuronCores, SPMD):
  - Owner-partition the 200k output rows: core c owns rows [c*25000,(c+1)*25000),
    split into 4 blocks of 6250; every edge is routed on the host to the core
    owning its destination.
  - All gathers use the NON-transpose dma_gather path so they can be spread
    across all 4 SWDGE queues (the transpose path's XBAR state races between
    queues; non-transpose is safe -- verified on HW). Calls are capped at 896
    indices (the descriptor-ring limit).
  - x is staged bf16; gathered rows land partition-major [128, n/128, 256];
    each 128-edge tile is transposed on the TensorEngine (identity matmul)
    and the form-A GEMM accumulates the two C_in K-tiles into PSUM.
  - Messages are written (bf16) to a per-block DRAM buffer laid out in
    gather-row order; the scatter-add is realized race-free as "rounds":
    the host sorts each block's rows by message count, round r adds the r-th
    message of every row that has one -- a 128-aligned contiguous prefix of
    the block's accumulator slots -- via a non-transpose gather + DVE add.
  - The accumulator stays fully SBUF-resident (row-major [128, 196, 128]).
    BatchNorm stats: per-slot square (ACT) + strided DVE reduces, then a
    TensorEngine ones-matmul partition-reduce, a [1,256] AllReduce (sync-BN),
    outer-product broadcast of scale/bias, and an in-place normalize+ReLU.
  - Host inverts the count-sort permutation and row-major layout during
    unsharding.
"""
import sys

sys.path.insert(0, "/opt/trn_rl_repo")

import numpy as np
import ml_dtypes

from concourse import bass, mybir, bacc
from concourse import tile
from concourse.bass_utils import run_bass_kernel_spmd

F32 = mybir.dt.float32
BF16 = mybir.dt.bfloat16
I16 = mybir.dt.int16

N_IN = 100000
N_OUT = 200000
C_IN = 256
C_OUT = 128
K = 8
NC = 8
RPC = N_OUT // NC          # rows per core = 25000
B = 4                      # blocks per core
RPB = RPC // B             # rows per block = 6250
XCH = 4                    # x chunks (int16 gather index limit)
XCHSZ = N_IN // XCH        # 25000
SLOTB = 49                 # acc slots per block (49*128 = 6272 >= RPB)
NSLOT = B * SLOTB          # 196
BN_EPS = 1e-5
GCHUNK = 896               # max indices per dma_gather call
NQ = 4                     # SWDGE queues

LAST_EXEC_NS = [None]


def _ceil(a, b):
    return (a + b - 1) // b


def _wrap_idx(arr):
    """[n] -> [128, n/16] wrapped+replicated layout for dma_gather."""
    n = arr.shape[0]
    assert n % 16 == 0
    w = arr.reshape(n // 16, 16).T.astype(np.int16)
    return np.tile(w, (8, 1))


def _preprocess(in_map, out_map):
    im = in_map.reshape(-1).astype(np.int64)
    dm = out_map.reshape(-1).astype(np.int64)
    ne = im.shape[0]
    kk = np.arange(ne, dtype=np.int64) // N_IN

    core = dm // RPC
    loc = dm - core * RPC
    blk = loc // RPB
    row = loc - blk * RPB
    xch = im // XCHSZ
    xli = im - xch * XCHSZ

    gid = ((core * B + blk) * XCH + xch) * K + kk
    order = np.argsort(gid, kind="stable")
    gcnt = np.bincount(gid, minlength=NC * B * XCH * K).reshape(NC, B, XCH, K)

    S = (_ceil(np.maximum(gcnt.max(axis=0), 1), 128) * 128).astype(np.int64)  # [B,XCH,K]
    goff = np.zeros((B, XCH, K), np.int64)
    for b in range(B):
        off = 0
        for c4 in range(XCH):
            for k in range(K):
                goff[b, c4, k] = off
                off += S[b, c4, k]
    GB = S.reshape(B, -1).sum(axis=1)
    T = [int(GB[b]) // 128 for b in range(B)]      # msg tiles per block
    assert all(128 * (t + 1) <= 32767 for t in T), "msg row ids exceed int16"
    Tmax = max(T)

    im_s = xli[order]
    row_s = row[order]
    core_s = core[order]
    blk_s = blk[order]
    gid_s = gid[order]

    grp_start = np.zeros(NC * B * XCH * K + 1, np.int64)
    np.cumsum(np.bincount(gid_s, minlength=NC * B * XCH * K), out=grp_start[1:])
    pos_in_grp = np.arange(ne) - grp_start[gid_s]

    c4_of = (gid_s // K) % XCH
    k_of = gid_s % K
    pos = goff[blk_s, c4_of, k_of] + pos_in_grp          # position in block stream
    # msg DRAM row id: [p, t] layout -> row = p*(T+1) + t
    Tb = np.array(T)[blk_s]
    msgrow = (pos % 128) * (Tb + 1) + pos // 128

    # ---- rounds ---------------------------------------------------------
    cb = core_s * B + blk_s
    counts = np.zeros((NC * B, RPB), np.int64)
    np.add.at(counts, (cb, row_s), 1)
    maxcnt = int(counts.max())
    cnt_sorted = -np.sort(-counts, axis=1)
    n_r = np.zeros((NC * B, maxcnt), np.int64)
    for r in range(maxcnt):
        n_r[:, r] = (cnt_sorted > r).sum(axis=1)
    NR = n_r.reshape(NC, B, maxcnt).max(axis=0)          # [B, maxcnt]
    # pad each round to a 128 multiple (adds are slot-aligned)
    ROUNDS = [[int(_ceil(NR[b, r], 128) * 128) for r in range(maxcnt) if NR[b, r] > 0]
              for b in range(B)]
    RT = [max(sum(rs), 128) for rs in ROUNDS]            # mult of 128
    RTmax = max(RT)

    rank_of_row = np.zeros((NC * B, RPB), np.int64)
    rows_sorted = np.argsort(-counts, axis=1, kind="stable")
    for i in range(NC * B):
        rank_of_row[i, rows_sorted[i]] = np.arange(RPB)

    key = cb * RPB + row_s
    okey = np.argsort(key, kind="stable")
    kstart = np.zeros(NC * B * RPB + 1, np.int64)
    np.cumsum(np.bincount(key, minlength=NC * B * RPB), out=kstart[1:])
    occ = np.empty(ne, np.int64)
    occ[okey] = np.arange(ne) - kstart[key[okey]]

    r_off = [np.concatenate([[0], np.cumsum(rs)]).astype(np.int64) for rs in ROUNDS]

    gidx_cores, ridx_cores = [], []
    GBmax = int(GB.max())
    for c in range(NC):
        gidx = np.zeros((B, GBmax), np.int64)
        sel0 = core_s == c
        gidx[blk_s[sel0], pos[sel0]] = im_s[sel0]
        # ridx default = zero row (id T[b]) per block
        ridx = np.empty((B, RTmax), np.int64)
        for b in range(B):
            ridx[b, :] = T[b]
        roffs = np.array([[r_off[b][r] if r < len(ROUNDS[b]) else 0
                           for r in range(maxcnt + 1)] for b in range(B)])
        bsel = blk_s[sel0]
        qsel = rank_of_row[cb[sel0], row_s[sel0]]
        rsel = occ[sel0]
        ridx[bsel, roffs[bsel, rsel] + qsel] = msgrow[sel0]
        gidx_cores.append(np.concatenate([_wrap_idx(gidx[b]) for b in range(B)], axis=1))
        ridx_cores.append(np.concatenate([_wrap_idx(ridx[b]) for b in range(B)], axis=1))

    # per-block tile index -> k
    tile_k = []
    for b in range(B):
        tk = []
        for c4 in range(XCH):
            for k in range(K):
                tk += [k] * (int(S[b, c4, k]) // 128)
        tile_k.append(tk)

    meta = dict(S=S, goff=goff, GB=GB, T=T, Tmax=Tmax, ROUNDS=ROUNDS, RT=RT,
                RTmax=RTmax, rows_sorted=rows_sorted, maxcnt=maxcnt, tile_k=tile_k)
    return gidx_cores, ridx_cores, meta


def _build(meta, sim_mode=False):
    S, goff, GB, T = meta["S"], meta["goff"], meta["GB"], meta["T"]
    ROUNDS, RT, RTmax = meta["ROUNDS"], meta["RT"], meta["RTmax"]
    tile_k = meta["tile_k"]
    GBmax = int(GB.max())

    nc = bacc.Bacc(num_devices=NC, num_swdge_queues=NQ)
    x_d = nc.dram_tensor("x", [N_IN, C_IN], BF16, kind="ExternalInput")
    w_d = nc.dram_tensor("w", [128, 2 * K, C_OUT], BF16, kind="ExternalInput")
    gidx_d = nc.dram_tensor("gidx", [128, B * GBmax // 16], I16, kind="ExternalInput")
    ridx_d = nc.dram_tensor("ridx", [128, B * RTmax // 16], I16, kind="ExternalInput")
    gamma_d = nc.dram_tensor("gamma", [1, C_OUT], F32, kind="ExternalInput")
    beta_d = nc.dram_tensor("beta", [1, C_OUT], F32, kind="ExternalInput")
    ident_d = nc.dram_tensor("ident", [128, 128], BF16, kind="ExternalInput")
    onesc_d = nc.dram_tensor("onesc", [128, 1], F32, kind="ExternalInput")
    onesr_d = nc.dram_tensor("onesr", [1, 128], F32, kind="ExternalInput")
    out_d = nc.dram_tensor("out", [128, NSLOT, C_OUT], F32, kind="ExternalOutput")
    msgs_d = [nc.dram_tensor(f"msgs{b}", [128 * (T[b] + 1), C_OUT], BF16)
              for b in range(B)]
    cc_in = nc.dram_tensor("cc_in", [1, 256], F32)
    cc_out = nc.dram_tensor("cc_out", [1, 256], F32)

    with tile.TileContext(nc) as tc:
        with (
            tc.tile_pool(name="const", bufs=1) as cpool,
            tc.tile_pool(name="accp", bufs=1) as apool,
            tc.tile_pool(name="xgp", bufs=3) as xpool,
            tc.tile_pool(name="ltp", bufs=8) as lpool,
            tc.tile_pool(name="stg", bufs=3) as spool,
            tc.tile_pool(name="idxp", bufs=2) as ipool,
            tc.tile_pool(name="rgp", bufs=4) as rpool,
            tc.tile_pool(name="sqp", bufs=3) as qpool,
            tc.tile_pool(name="psT", bufs=2, space="PSUM") as psT,
            tc.tile_pool(name="psG", bufs=3, space="PSUM") as psG,
            tc.tile_pool(name="psS", bufs=1, space="PSUM") as psS,
        ):
            w_sb = cpool.tile([128, 2 * K, C_OUT], BF16)
            nc.sync.dma_start(w_sb[:], w_d[:])
            ident = cpool.tile([128, 128], BF16)
            nc.sync.dma_start(ident[:], ident_d[:])
            gamma_sb = cpool.tile([1, C_OUT], F32)
            nc.sync.dma_start(gamma_sb[:], gamma_d[:])
            beta_sb = cpool.tile([1, C_OUT], F32)
            nc.sync.dma_start(beta_sb[:], beta_d[:])
            onesc = cpool.tile([128, 1], F32)
            nc.sync.dma_start(onesc[:], onesc_d[:])
            onesr = cpool.tile([1, 128], F32)
            nc.sync.dma_start(onesr[:], onesr_d[:])

            acc = apool.tile([128, NSLOT, C_OUT], F32)
            nc.vector.memset(acc[:], 0.0)
            ssum = cpool.tile([128, C_OUT], F32)
            nc.vector.memset(ssum[:], 0.0)
            ssq = cpool.tile([128, C_OUT], F32)
            nc.vector.memset(ssq[:], 0.0)

            for b in range(B):
                gidx_sb = ipool.tile([128, GBmax // 16], I16, tag="gidx")
                nc.sync.dma_start(
                    gidx_sb[:], gidx_d[:, b * (GBmax // 16):(b + 1) * (GBmax // 16)]
                )
                ridx_sb = ipool.tile([128, RTmax // 16], I16, tag="ridx")
                nc.sync.dma_start(
                    ridx_sb[:], ridx_d[:, b * (RTmax // 16):(b + 1) * (RTmax // 16)]
                )
                msgv = msgs_d[b][:].rearrange("(p t) c -> p t c", p=128)
                zrow = spool.tile([128, 1, C_OUT], BF16, tag="zr")
                nc.vector.memset(zrow[:], 0.0)
                nc.sync.dma_start(msgv[:, T[b]:T[b] + 1, :], zrow[:])

                # ---- gather + transpose + GEMM -> msgs DRAM -----------
                for c4 in range(XCH):
                    xsrc = x_d[c4 * XCHSZ:(c4 + 1) * XCHSZ, :]
                    seg_off = int(goff[b, c4, 0])
                    seg_len = int(S[b, c4, :].sum())
                    p0 = 0
                    while p0 < seg_len:
                        ln = min(GCHUNK, seg_len - p0)
                        nt = ln // 128
                        off = seg_off + p0
                        xg = xpool.tile([128, nt, C_IN], BF16, tag="xg")
                        nc.gpsimd.dma_gather(
                            out_ap=xg[:],
                            in_ap=xsrc,
                            idxs_ap=gidx_sb[:, off // 16:(off + ln) // 16],
                            num_idxs=ln,
                            num_idxs_reg=ln,
                            elem_size=C_IN,
                            transpose=False,
                        )
                        stag = spool.tile([128, nt, C_OUT], BF16, tag="stag")
                        t = 0
                        while t < nt:
                            ng = min(4, nt - t)
                            pst = psT.tile([128, 512], BF16, tag="psT")
                            psg = psG.tile([128, 512], F32, tag="psG")
                            lt = lpool.tile([128, ng, 256], BF16, tag="lt")
                            for j in range(ng):
                                nc.tensor.transpose(
                                    pst[:, j * 128:(j + 1) * 128],
                                    xg[:, t + j, 0:128], ident[:])
                            nc.vector.tensor_copy(
                                lt[:, :ng, 0:128],
                                pst[:, :ng * 128].rearrange("p (g c) -> p g c", c=128))
                            pst2 = psT.tile([128, 512], BF16, tag="psT")
                            for j in range(ng):
                                nc.tensor.transpose(
                                    pst2[:, j * 128:(j + 1) * 128],
                                    xg[:, t + j, 128:256], ident[:])
                            nc.vector.tensor_copy(
                                lt[:, :ng, 128:256],
                                pst2[:, :ng * 128].rearrange("p (g c) -> p g c", c=128))
                            for j in range(ng):
                                gt = off // 128 + t + j
                                k = tile_k[b][gt]
                                nc.tensor.matmul(
                                    psg[:, j * 128:(j + 1) * 128],
                                    lt[:, j, 0:128], w_sb[:, 2 * k, :],
                                    start=True, stop=False)
                                nc.tensor.matmul(
                                    psg[:, j * 128:(j + 1) * 128],
                                    lt[:, j, 128:256], w_sb[:, 2 * k + 1, :],
                                    start=False, stop=True)
                            nc.scalar.copy(
                                stag[:, t:t + ng, :],
                                psg[:, :ng * 128].rearrange("p (g c) -> p g c", c=128))
                            t += ng
                        nc.sync.dma_start(
                            msgv[:, off // 128:off // 128 + nt, :], stag[:])
                        p0 += ln

                # ---- rounds: gather + prefix adds ---------------------
                segs = []
                src = 0
                for n in ROUNDS[b]:
                    segs.append((src, n))
                    src += n
                total = RT[b]
                gpos = 0
                while gpos < total:
                    gl = min(GCHUNK, total - gpos)
                    g = rpool.tile([128, gl // 128, C_OUT], BF16, tag="rg")
                    nc.gpsimd.dma_gather(
                        out_ap=g[:],
                        in_ap=msgs_d[b][:],
                        idxs_ap=ridx_sb[:, gpos // 16:(gpos + gl) // 16],
                        num_idxs=gl,
                        num_idxs_reg=gl,
                        elem_size=C_OUT,
                        transpose=False,
                    )
                    for (soff, slen) in segs:
                        lo = max(soff, gpos)
                        hi = min(soff + slen, gpos + gl)
                        if lo >= hi:
                            continue
                        s0 = b * SLOTB + (lo - soff) // 128
                        s1 = b * SLOTB + (hi - soff) // 128
                        nc.vector.tensor_tensor(
                            acc[:, s0:s1, :],
                            acc[:, s0:s1, :],
                            g[:, (lo - gpos) // 128:(hi - gpos) // 128, :],
                            op=mybir.AluOpType.add,
                        )
                    gpos += gl

            # ---- BN stats --------------------------------------------
            accv = acc[:].rearrange("p s c -> p c s")
            nc.vector.reduce_sum(ssum[:], accv, mybir.AxisListType.X)
            for s in range(NSLOT):
                sq = qpool.tile([128, C_OUT], F32, tag="sq")
                nc.scalar.square(sq[:], acc[:, s, :])
                nc.vector.tensor_tensor(ssq[:], ssq[:], sq[:], op=mybir.AluOpType.add)

            pss = psS.tile([1, 128], F32, tag="pss")
            nc.tensor.matmul(pss[:], onesc[:], ssum[:], start=True, stop=True)
            pss2 = psS.tile([1, 128], F32, tag="pss")
            nc.tensor.matmul(pss2[:], onesc[:], ssq[:], start=True, stop=True)
            st = cpool.tile([1, 256], F32)
            nc.vector.tensor_copy(st[:, 0:128], pss[:])
            nc.vector.tensor_copy(st[:, 128:256], pss2[:])
            nc.sync.dma_start(cc_in[:], st[:])
            if sim_mode:
                nc.sync.dma_start(cc_out[:], cc_in[:])
            else:
                nc.gpsimd.collective_compute(
                    "AllReduce", mybir.AluOpType.add,
                    replica_groups=[list(range(NC))],
                    ins=[cc_in[:]], outs=[cc_out[:]],
                )
            st2 = cpool.tile([1, 256], F32)
            nc.sync.dma_start(st2[:], cc_out[:])

            mean = cpool.tile([1, 128], F32)
            nc.scalar.mul(mean[:], st2[:, 0:128], 1.0 / N_OUT)
            e2 = cpool.tile([1, 128], F32)
            nc.scalar.mul(e2[:], st2[:, 128:256], 1.0 / N_OUT)
            m2 = cpool.tile([1, 128], F32)
            nc.scalar.square(m2[:], mean[:])
            var = cpool.tile([1, 128], F32)
            nc.vector.tensor_sub(var[:], e2[:], m2[:])
            eps_sb = cpool.tile([1, 1], F32)
            nc.vector.memset(eps_sb[:], BN_EPS)
            std = cpool.tile([1, 128], F32)
            nc.scalar.activation(std[:], var[:], mybir.ActivationFunctionType.Sqrt,
                                 bias=eps_sb[:], scale=1.0)
            inv = cpool.tile([1, 128], F32)
            nc.vector.reciprocal(inv[:], std[:])
            scl = cpool.tile([1, 128], F32)
            nc.vector.tensor_mul(scl[:], inv[:], gamma_sb[:])
            ms = cpool.tile([1, 128], F32)
            nc.vector.tensor_mul(ms[:], mean[:], scl[:])
            bia = cpool.tile([1, 128], F32)
            nc.vector.tensor_sub(bia[:], beta_sb[:], ms[:])

            # broadcast scale/bias to [128, 128] via PE outer product
            psb = psS.tile([128, 128], F32, tag="psb")
            nc.tensor.matmul(psb[:], onesr[:], scl[:], start=True, stop=True)
            sclB = cpool.tile([128, 128], F32)
            nc.vector.tensor_copy(sclB[:], psb[:])
            psb2 = psS.tile([128, 128], F32, tag="psb")
            nc.tensor.matmul(psb2[:], onesr[:], bia[:], start=True, stop=True)
            biaB = cpool.tile([128, 128], F32)
            nc.vector.tensor_copy(biaB[:], psb2[:])

            # ---- normalize + ReLU (in place) + store ------------------
            for s in range(NSLOT):
                a = acc[:, s, :]
                nc.vector.tensor_mul(a, a, sclB[:])
                nc.vector.tensor_add(a, a, biaB[:])
                nc.vector.tensor_scalar_max(a, a, 0.0)
            nc.sync.dma_start(out_d[:], acc[:])

    # Route each SWDGE gather to the queue matching its Tile-assigned DMASW
    # lane (sem lane i is claimed by queue i % NQ).
    from concourse.tile_sem_assignment import PROC_NAME_TO_IDX
    dmasw = {PROC_NAME_TO_IDX[f"DMASW{i}"]: i for i in range(8)}
    for ins in nc.inst_map.values():
        if isinstance(ins, mybir.InstDMAGatherAnt):
            proc = getattr(ins, "bass_scheduled_proc", None)
            if proc in dmasw:
                ins.queue_num = dmasw[proc] % NQ

    nc.compile()
    return nc


def kernel(x_feats, weight, gamma, beta, in_map, out_map, n_out, _trace=False):
    assert int(n_out) == N_OUT
    gidx_cores, ridx_cores, meta = _preprocess(np.asarray(in_map), np.asarray(out_map))
    nc = _build(meta)

    xbf = np.asarray(x_feats, np.float32).astype(ml_dtypes.bfloat16)
    wbf = np.asarray(weight, np.float32).astype(ml_dtypes.bfloat16)
    wdev = np.ascontiguousarray(
        wbf.reshape(K, 2, 128, C_OUT).transpose(2, 0, 1, 3).reshape(128, 2 * K, C_OUT)
    )
    gdev = np.asarray(gamma, np.float32).reshape(1, C_OUT)
    bdev = np.asarray(beta, np.float32).reshape(1, C_OUT)
    ident = np.eye(128, dtype=np.float32).astype(ml_dtypes.bfloat16)
    onesc = np.ones((128, 1), np.float32)
    onesr = np.ones((1, 128), np.float32)

    in_maps = []
    for c in range(NC):
        in_maps.append({
            "x": xbf,
            "w": wdev,
            "gidx": np.ascontiguousarray(gidx_cores[c]),
            "ridx": np.ascontiguousarray(ridx_cores[c]),
            "gamma": gdev,
            "beta": bdev,
            "ident": ident,
            "onesc": onesc,
            "onesr": onesr,
        })

    kw = dict(trace=True) if _trace else {}
    res = run_bass_kernel_spmd(nc, in_maps, core_ids=list(range(NC)), **kw)
    LAST_EXEC_NS[0] = res.exec_time_ns

    out = np.empty((N_OUT, C_OUT), np.float32)
    rows_sorted = meta["rows_sorted"]
    for c in range(NC):
        y = res.results[c]["out"]  # [128, NSLOT, C_OUT]
        for b in range(B):
            vals = y[:, b * SLOTB:(b + 1) * SLOTB, :]          # [128, 49, C]
            flat = vals.transpose(1, 0, 2).reshape(SLOTB * 128, C_OUT)[:RPB]
            rows = c * RPC + b * RPB + rows_sorted[c * B + b]
            out[rows] = flat
    return out


# revision 17
# speedup vs baseline: 1.9328x; 1.1311x over previous
"""Distributed Trainium2 kernel for MinkowskiEngine-style sparse transposed
conv + BatchNorm + ReLU (gather -> per-offset GEMM -> scatter-add -> BN -> ReLU).

v2 strategy (8 NeuronCores, SPMD):
  - Owner-partition the 200k output rows: core c owns rows [c*25000,(c+1)*25000),
    split into 4 blocks of 6250; every edge is routed on the host to the core
    owning its destination.
  - All gathers use the NON-transpose dma_gather path so they can be spread
    across all 4 SWDGE queues (the transpose path's XBAR state races between
    queues; non-transpose is safe -- verified on HW). Calls are capped at 896
    indices (the descriptor-ring limit).
  - x is staged bf16; gathered rows land partition-major [128, n/128, 256];
    each 128-edge tile is transposed on the TensorEngine (identity matmul)
    and the form-A GEMM accumulates the two C_in K-tiles into PSUM.
  - Messages are written (bf16) to a per-block DRAM buffer laid out in
    gather-row order; the scatter-add is realized race-free as "rounds":
    the host sorts each block's rows by message count, round r adds the r-th
    message of every row that has one -- a 128-aligned contiguous prefix of
    the block's accumulator slots -- via a non-transpose gather + DVE add.
  - The accumulator stays fully SBUF-resident (row-major [128, 196, 128]).
    BatchNorm stats: per-slot square (ACT) + strided DVE reduces, then a
    TensorEngine ones-matmul partition-reduce, a [1,256] AllReduce (sync-BN),
    outer-product broadcast of scale/bias, and an in-place normalize+ReLU.
  - Host inverts the count-sort permutation and row-major layout during
    unsharding.
"""
import sys

sys.path.insert(0, "/opt/trn_rl_repo")

import numpy as np
import ml_dtypes

from concourse import bass, mybir, bacc
from concourse import tile
from concourse.bass_utils import run_bass_kernel_spmd

F32 = mybir.dt.float32
BF16 = mybir.dt.bfloat16
I16 = mybir.dt.int16

N_IN = 100000
N_OUT = 200000
C_IN = 256
C_OUT = 128
K = 8
NC = 8
RPC = N_OUT // NC          # rows per core = 25000
B = 4                      # blocks per core
RPB = RPC // B             # rows per block = 6250
XCH = 4                    # x chunks (int16 gather index limit)
XCHSZ = N_IN // XCH        # 25000
SLOTB = 49                 # acc slots per block (49*128 = 6272 >= RPB)
NSLOT = B * SLOTB          # 196
BN_EPS = 1e-5
GCHUNK = 896               # max indices per dma_gather call
NQ = 4                     # SWDGE queues

LAST_EXEC_NS = [None]


def _ceil(a, b):
    return (a + b - 1) // b


def _wrap_idx(arr):
    """[n] -> [128, n/16] wrapped+replicated layout for dma_gather."""
    n = arr.shape[0]
    assert n % 16 == 0
    w = arr.reshape(n // 16, 16).T.astype(np.int16)
    return np.tile(w, (8, 1))


def _preprocess(in_map, out_map):
    im = in_map.reshape(-1).astype(np.int64)
    dm = out_map.reshape(-1).astype(np.int64)
    ne = im.shape[0]
    kk = np.arange(ne, dtype=np.int64) // N_IN

    core = dm // RPC
    loc = dm - core * RPC
    blk = loc // RPB
    row = loc - blk * RPB
    xch = im // XCHSZ
    xli = im - xch * XCHSZ

    gid = ((core * B + blk) * XCH + xch) * K + kk
    order = np.argsort(gid, kind="stable")
    gcnt = np.bincount(gid, minlength=NC * B * XCH * K).reshape(NC, B, XCH, K)

    S = (_ceil(np.maximum(gcnt.max(axis=0), 1), 128) * 128).astype(np.int64)  # [B,XCH,K]
    goff = np.zeros((B, XCH, K), np.int64)
    for b in range(B):
        off = 0
        for c4 in range(XCH):
            for k in range(K):
                goff[b, c4, k] = off
                off += S[b, c4, k]
    GB = S.reshape(B, -1).sum(axis=1)
    T = [int(GB[b]) // 128 for b in range(B)]      # msg tiles per block
    assert all(128 * (t + 1) <= 32767 for t in T), "msg row ids exceed int16"
    Tmax = max(T)

    im_s = xli[order]
    row_s = row[order]
    core_s = core[order]
    blk_s = blk[order]
    gid_s = gid[order]

    grp_start = np.zeros(NC * B * XCH * K + 1, np.int64)
    np.cumsum(np.bincount(gid_s, minlength=NC * B * XCH * K), out=grp_start[1:])
    pos_in_grp = np.arange(ne) - grp_start[gid_s]

    c4_of = (gid_s // K) % XCH
    k_of = gid_s % K
    pos = goff[blk_s, c4_of, k_of] + pos_in_grp          # position in block stream
    # msg DRAM row id: [p, t] layout -> row = p*(T+1) + t
    Tb = np.array(T)[blk_s]
    msgrow = (pos % 128) * (Tb + 1) + pos // 128

    # ---- rounds ---------------------------------------------------------
    cb = core_s * B + blk_s
    counts = np.zeros((NC * B, RPB), np.int64)
    np.add.at(counts, (cb, row_s), 1)
    maxcnt = int(counts.max())
    cnt_sorted = -np.sort(-counts, axis=1)
    n_r = np.zeros((NC * B, maxcnt), np.int64)
    for r in range(maxcnt):
        n_r[:, r] = (cnt_sorted > r).sum(axis=1)
    NR = n_r.reshape(NC, B, maxcnt).max(axis=0)          # [B, maxcnt]
    # pad each round to a 128 multiple (adds are slot-aligned)
    ROUNDS = [[int(_ceil(NR[b, r], 128) * 128) for r in range(maxcnt) if NR[b, r] > 0]
              for b in range(B)]
    RT = [max(sum(rs), 128) for rs in ROUNDS]            # mult of 128
    RTmax = max(RT)

    rank_of_row = np.zeros((NC * B, RPB), np.int64)
    rows_sorted = np.argsort(-counts, axis=1, kind="stable")
    for i in range(NC * B):
        rank_of_row[i, rows_sorted[i]] = np.arange(RPB)

    key = cb * RPB + row_s
    okey = np.argsort(key, kind="stable")
    kstart = np.zeros(NC * B * RPB + 1, np.int64)
    np.cumsum(np.bincount(key, minlength=NC * B * RPB), out=kstart[1:])
    occ = np.empty(ne, np.int64)
    occ[okey] = np.arange(ne) - kstart[key[okey]]

    r_off = [np.concatenate([[0], np.cumsum(rs)]).astype(np.int64) for rs in ROUNDS]

    gidx_cores, ridx_cores = [], []
    GBmax = int(GB.max())
    for c in range(NC):
        gidx = np.zeros((B, GBmax), np.int64)
        sel0 = core_s == c
        gidx[blk_s[sel0], pos[sel0]] = im_s[sel0]
        # ridx default = zero row (id T[b]) per block
        ridx = np.empty((B, RTmax), np.int64)
        for b in range(B):
            ridx[b, :] = T[b]
        roffs = np.array([[r_off[b][r] if r < len(ROUNDS[b]) else 0
                           for r in range(maxcnt + 1)] for b in range(B)])
        bsel = blk_s[sel0]
        qsel = rank_of_row[cb[sel0], row_s[sel0]]
        rsel = occ[sel0]
        ridx[bsel, roffs[bsel, rsel] + qsel] = msgrow[sel0]
        gidx_cores.append(np.concatenate([_wrap_idx(gidx[b]) for b in range(B)], axis=1))
        ridx_cores.append(np.concatenate([_wrap_idx(ridx[b]) for b in range(B)], axis=1))

    # per-block tile index -> k
    tile_k = []
    for b in range(B):
        tk = []
        for c4 in range(XCH):
            for k in range(K):
                tk += [k] * (int(S[b, c4, k]) // 128)
        tile_k.append(tk)

    meta = dict(S=S, goff=goff, GB=GB, T=T, Tmax=Tmax, ROUNDS=ROUNDS, RT=RT,
                RTmax=RTmax, rows_sorted=rows_sorted, maxcnt=maxcnt, tile_k=tile_k)
    return gidx_cores, ridx_cores, meta


def _build(meta, sim_mode=False):
    S, goff, GB, T = meta["S"], meta["goff"], meta["GB"], meta["T"]
    ROUNDS, RT, RTmax = meta["ROUNDS"], meta["RT"], meta["RTmax"]
    tile_k = meta["tile_k"]
    GBmax = int(GB.max())

    nc = bacc.Bacc(num_devices=NC, num_swdge_queues=NQ)
    x_d = nc.dram_tensor("x", [N_IN, C_IN], BF16, kind="ExternalInput")
    w_d = nc.dram_tensor("w", [128, 2 * K, C_OUT], BF16, kind="ExternalInput")
    gidx_d = nc.dram_tensor("gidx", [128, B * GBmax // 16], I16, kind="ExternalInput")
    ridx_d = nc.dram_tensor("ridx", [128, B * RTmax // 16], I16, kind="ExternalInput")
    gamma_d = nc.dram_tensor("gamma", [1, C_OUT], F32, kind="ExternalInput")
    beta_d = nc.dram_tensor("beta", [1, C_OUT], F32, kind="ExternalInput")
    ident_d = nc.dram_tensor("ident", [128, 128], BF16, kind="ExternalInput")
    onesc_d = nc.dram_tensor("onesc", [128, 1], F32, kind="ExternalInput")
    onesr_d = nc.dram_tensor("onesr", [1, 128], F32, kind="ExternalInput")
    out_d = nc.dram_tensor("out", [128, NSLOT, C_OUT], F32, kind="ExternalOutput")
    msgs_d = [nc.dram_tensor(f"msgs{b}", [128 * (T[b] + 1), C_OUT], BF16)
              for b in range(B)]
    cc_in = nc.dram_tensor("cc_in", [1, 256], F32)
    cc_out = nc.dram_tensor("cc_out", [1, 256], F32)

    with tile.TileContext(nc) as tc:
        with (
            tc.tile_pool(name="const", bufs=1) as cpool,
            tc.tile_pool(name="accp", bufs=1) as apool,
            tc.tile_pool(name="xgp", bufs=3) as xpool,
            tc.tile_pool(name="ltp", bufs=8) as lpool,
            tc.tile_pool(name="stg", bufs=3) as spool,
            tc.tile_pool(name="idxp", bufs=2) as ipool,
            tc.tile_pool(name="rgp", bufs=4) as rpool,
            tc.tile_pool(name="sqp", bufs=3) as qpool,
            tc.tile_pool(name="psT", bufs=2, space="PSUM") as psT,
            tc.tile_pool(name="psG", bufs=3, space="PSUM") as psG,
            tc.tile_pool(name="psS", bufs=1, space="PSUM") as psS,
        ):
            w_sb = cpool.tile([128, 2 * K, C_OUT], BF16)
            nc.sync.dma_start(w_sb[:], w_d[:])
            ident = cpool.tile([128, 128], BF16)
            nc.sync.dma_start(ident[:], ident_d[:])
            gamma_sb = cpool.tile([1, C_OUT], F32)
            nc.sync.dma_start(gamma_sb[:], gamma_d[:])
            beta_sb = cpool.tile([1, C_OUT], F32)
            nc.sync.dma_start(beta_sb[:], beta_d[:])
            onesc = cpool.tile([128, 1], F32)
            nc.sync.dma_start(onesc[:], onesc_d[:])
            onesr = cpool.tile([1, 128], F32)
            nc.sync.dma_start(onesr[:], onesr_d[:])

            acc = apool.tile([128, NSLOT, C_OUT], F32)
            nc.vector.memset(acc[:], 0.0)
            ssum = cpool.tile([128, C_OUT], F32)
            nc.vector.memset(ssum[:], 0.0)
            ssq = cpool.tile([128, C_OUT], F32)
            nc.vector.memset(ssq[:], 0.0)

            def gemm_phase(b):
                gidx_sb = ipool.tile([128, GBmax // 16], I16, tag="gidx")
                nc.sync.dma_start(
                    gidx_sb[:], gidx_d[:, b * (GBmax // 16):(b + 1) * (GBmax // 16)]
                )
                msgv = msgs_d[b][:].rearrange("(p t) c -> p t c", p=128)
                zrow = spool.tile([128, 1, C_OUT], BF16, tag="zr")
                nc.vector.memset(zrow[:], 0.0)
                nc.sync.dma_start(msgv[:, T[b]:T[b] + 1, :], zrow[:])

                for c4 in range(XCH):
                    xsrc = x_d[c4 * XCHSZ:(c4 + 1) * XCHSZ, :]
                    seg_off = int(goff[b, c4, 0])
                    seg_len = int(S[b, c4, :].sum())
                    p0 = 0
                    while p0 < seg_len:
                        ln = min(GCHUNK, seg_len - p0)
                        nt = ln // 128
                        off = seg_off + p0
                        xg = xpool.tile([128, nt, C_IN], BF16, tag="xg")
                        nc.gpsimd.dma_gather(
                            out_ap=xg[:],
                            in_ap=xsrc,
                            idxs_ap=gidx_sb[:, off // 16:(off + ln) // 16],
                            num_idxs=ln,
                            num_idxs_reg=ln,
                            elem_size=C_IN,
                            transpose=False,
                        )
                        stag = spool.tile([128, nt, C_OUT], BF16, tag="stag")
                        t = 0
                        while t < nt:
                            ng = min(4, nt - t)
                            pst = psT.tile([128, 512], BF16, tag="psT")
                            psg = psG.tile([128, 512], F32, tag="psG")
                            lt = lpool.tile([128, ng, 256], BF16, tag="lt")
                            for j in range(ng):
                                nc.tensor.transpose(
                                    pst[:, j * 128:(j + 1) * 128],
                                    xg[:, t + j, 0:128], ident[:])
                            nc.vector.tensor_copy(
                                lt[:, :ng, 0:128],
                                pst[:, :ng * 128].rearrange("p (g c) -> p g c", c=128))
                            pst2 = psT.tile([128, 512], BF16, tag="psT")
                            for j in range(ng):
                                nc.tensor.transpose(
                                    pst2[:, j * 128:(j + 1) * 128],
                                    xg[:, t + j, 128:256], ident[:])
                            nc.vector.tensor_copy(
                                lt[:, :ng, 128:256],
                                pst2[:, :ng * 128].rearrange("p (g c) -> p g c", c=128))
                            for j in range(ng):
                                gt = off // 128 + t + j
                                k = tile_k[b][gt]
                                nc.tensor.matmul(
                                    psg[:, j * 128:(j + 1) * 128],
                                    lt[:, j, 0:128], w_sb[:, 2 * k, :],
                                    start=True, stop=False)
                                nc.tensor.matmul(
                                    psg[:, j * 128:(j + 1) * 128],
                                    lt[:, j, 128:256], w_sb[:, 2 * k + 1, :],
                                    start=False, stop=True)
                            nc.scalar.copy(
                                stag[:, t:t + ng, :],
                                psg[:, :ng * 128].rearrange("p (g c) -> p g c", c=128))
                            t += ng
                        nc.sync.dma_start(
                            msgv[:, off // 128:off // 128 + nt, :], stag[:])
                        p0 += ln

            def rounds_phase(b):
                ridx_sb = ipool.tile([128, RTmax // 16], I16, tag="ridx")
                nc.sync.dma_start(
                    ridx_sb[:], ridx_d[:, b * (RTmax // 16):(b + 1) * (RTmax // 16)]
                )
                segs = []
                src = 0
                for n in ROUNDS[b]:
                    segs.append((src, n))
                    src += n
                total = RT[b]
                gpos = 0
                while gpos < total:
                    gl = min(GCHUNK, total - gpos)
                    g = rpool.tile([128, gl // 128, C_OUT], BF16, tag="rg")
                    nc.gpsimd.dma_gather(
                        out_ap=g[:],
                        in_ap=msgs_d[b][:],
                        idxs_ap=ridx_sb[:, gpos // 16:(gpos + gl) // 16],
                        num_idxs=gl,
                        num_idxs_reg=gl,
                        elem_size=C_OUT,
                        transpose=False,
                    )
                    for (soff, slen) in segs:
                        lo = max(soff, gpos)
                        hi = min(soff + slen, gpos + gl)
                        if lo >= hi:
                            continue
                        s0 = b * SLOTB + (lo - soff) // 128
                        s1 = b * SLOTB + (hi - soff) // 128
                        nc.vector.tensor_tensor(
                            acc[:, s0:s1, :],
                            acc[:, s0:s1, :],
                            g[:, (lo - gpos) // 128:(hi - gpos) // 128, :],
                            op=mybir.AluOpType.add,
                        )
                    gpos += gl
                # per-block partial stats (overlaps next block's gathers)
                for s in range(b * SLOTB, (b + 1) * SLOTB):
                    sq = qpool.tile([128, C_OUT], F32, tag="sq")
                    nc.scalar.square(sq[:], acc[:, s, :])
                    nc.vector.tensor_tensor(ssq[:], ssq[:], sq[:],
                                            op=mybir.AluOpType.add)

            # pipelined emission: G0 G1 R0 G2 R1 G3 R2 R3
            gemm_phase(0)
            gemm_phase(1)
            rounds_phase(0)
            gemm_phase(2)
            rounds_phase(1)
            gemm_phase(3)
            rounds_phase(2)
            rounds_phase(3)

            # ---- BN stats --------------------------------------------
            accv = acc[:].rearrange("p s c -> p c s")
            nc.vector.reduce_sum(ssum[:], accv, mybir.AxisListType.X)

            pss = psS.tile([1, 128], F32, tag="pss")
            nc.tensor.matmul(pss[:], onesc[:], ssum[:], start=True, stop=True)
            pss2 = psS.tile([1, 128], F32, tag="pss")
            nc.tensor.matmul(pss2[:], onesc[:], ssq[:], start=True, stop=True)
            st = cpool.tile([1, 256], F32)
            nc.vector.tensor_copy(st[:, 0:128], pss[:])
            nc.vector.tensor_copy(st[:, 128:256], pss2[:])
            nc.sync.dma_start(cc_in[:], st[:])
            if sim_mode:
                nc.sync.dma_start(cc_out[:], cc_in[:])
            else:
                nc.gpsimd.collective_compute(
                    "AllReduce", mybir.AluOpType.add,
                    replica_groups=[list(range(NC))],
                    ins=[cc_in[:]], outs=[cc_out[:]],
                )
            st2 = cpool.tile([1, 256], F32)
            nc.sync.dma_start(st2[:], cc_out[:])

            mean = cpool.tile([1, 128], F32)
            nc.scalar.mul(mean[:], st2[:, 0:128], 1.0 / N_OUT)
            e2 = cpool.tile([1, 128], F32)
            nc.scalar.mul(e2[:], st2[:, 128:256], 1.0 / N_OUT)
            m2 = cpool.tile([1, 128], F32)
            nc.scalar.square(m2[:], mean[:])
            var = cpool.tile([1, 128], F32)
            nc.vector.tensor_sub(var[:], e2[:], m2[:])
            eps_sb = cpool.tile([1, 1], F32)
            nc.vector.memset(eps_sb[:], BN_EPS)
            std = cpool.tile([1, 128], F32)
            nc.scalar.activation(std[:], var[:], mybir.ActivationFunctionType.Sqrt,
                                 bias=eps_sb[:], scale=1.0)
            inv = cpool.tile([1, 128], F32)
            nc.vector.reciprocal(inv[:], std[:])
            scl = cpool.tile([1, 128], F32)
            nc.vector.tensor_mul(scl[:], inv[:], gamma_sb[:])
            ms = cpool.tile([1, 128], F32)
            nc.vector.tensor_mul(ms[:], mean[:], scl[:])
            bia = cpool.tile([1, 128], F32)
            nc.vector.tensor_sub(bia[:], beta_sb[:], ms[:])

            # broadcast scale/bias to [128, 128] via PE outer product
            psb = psS.tile([128, 128], F32, tag="psb")
            nc.tensor.matmul(psb[:], onesr[:], scl[:], start=True, stop=True)
            sclB = cpool.tile([128, 128], F32)
            nc.vector.tensor_copy(sclB[:], psb[:])
            psb2 = psS.tile([128, 128], F32, tag="psb")
            nc.tensor.matmul(psb2[:], onesr[:], bia[:], start=True, stop=True)
            biaB = cpool.tile([128, 128], F32)
            nc.vector.tensor_copy(biaB[:], psb2[:])

            # ---- normalize + ReLU (in place) + store ------------------
            for s in range(NSLOT):
                a = acc[:, s, :]
                nc.vector.tensor_mul(a, a, sclB[:])
                nc.vector.tensor_add(a, a, biaB[:])
                nc.vector.tensor_scalar_max(a, a, 0.0)
            nc.sync.dma_start(out_d[:], acc[:])

    # Route each SWDGE gather to the queue matching its Tile-assigned DMASW
    # lane (sem lane i is claimed by queue i % NQ).
    from concourse.tile_sem_assignment import PROC_NAME_TO_IDX
    dmasw = {PROC_NAME_TO_IDX[f"DMASW{i}"]: i for i in range(8)}
    for ins in nc.inst_map.values():
        if isinstance(ins, mybir.InstDMAGatherAnt):
            proc = getattr(ins, "bass_scheduled_proc", None)
            if proc in dmasw:
                ins.queue_num = dmasw[proc] % NQ

    nc.compile()
    return nc


def kernel(x_feats, weight, gamma, beta, in_map, out_map, n_out, _trace=False):
    assert int(n_out) == N_OUT
    gidx_cores, ridx_cores, meta = _preprocess(np.asarray(in_map), np.asarray(out_map))
    nc = _build(meta)

    xbf = np.asarray(x_feats, np.float32).astype(ml_dtypes.bfloat16)
    wbf = np.asarray(weight, np.float32).astype(ml_dtypes.bfloat16)
    wdev = np.ascontiguousarray(
        wbf.reshape(K, 2, 128, C_OUT).transpose(2, 0, 1, 3).reshape(128, 2 * K, C_OUT)
    )
    gdev = np.asarray(gamma, np.float32).reshape(1, C_OUT)
    bdev = np.asarray(beta, np.float32).reshape(1, C_OUT)
    ident = np.eye(128, dtype=np.float32).astype(ml_dtypes.bfloat16)
    onesc = np.ones((128, 1), np.float32)
    onesr = np.ones((1, 128), np.float32)

    in_maps = []
    for c in range(NC):
        in_maps.append({
            "x": xbf,
            "w": wdev,
            "gidx": np.ascontiguousarray(gidx_cores[c]),
            "ridx": np.ascontiguousarray(ridx_cores[c]),
            "gamma": gdev,
            "beta": bdev,
            "ident": ident,
            "onesc": onesc,
            "onesr": onesr,
        })

    kw = dict(trace=True) if _trace else {}
    res = run_bass_kernel_spmd(nc, in_maps, core_ids=list(range(NC)), **kw)
    LAST_EXEC_NS[0] = res.exec_time_ns

    out = np.empty((N_OUT, C_OUT), np.float32)
    rows_sorted = meta["rows_sorted"]
    for c in range(NC):
        y = res.results[c]["out"]  # [128, NSLOT, C_OUT]
        for b in range(B):
            vals = y[:, b * SLOTB:(b + 1) * SLOTB, :]          # [128, 49, C]
            flat = vals.transpose(1, 0, 2).reshape(SLOTB * 128, C_OUT)[:RPB]
            rows = c * RPC + b * RPB + rows_sorted[c * B + b]
            out[rows] = flat
    return out
